# revision 32
# baseline (speedup 1.0000x reference)
"""Multi-head self-attention on 8 TRN2 NeuronCores.

Problem: x[2,2048,1024] -> qkv proj -> 16-head attention -> out proj.
Sharding: core c handles batch b=c//4 and head group g=c%4 (4 heads each).
Each core computes a partial output y_c[2048,1024] = attn_out_heads(g) @ W_proj[rows g];
host sums the 4 partials per batch and adds b_proj.

Design notes (from HW measurement, ~224 us/core vs 253 us fp32r baseline):
  - stage 1 (qkv projection) runs DENSE, back-to-back: spreading these
    matmuls thinly into the attention stream measured 1.5-2x slower on HW
    (PE HAM clock-gate drops to 1.2 GHz in sparse phases + cross-engine
    semaphore chains); only the q blocks for i 1024-2047 and the
    out-projection drip into the attention stream, as whole dense blocks.
  - input DMAs are few + big (one InstDMACopy splits over all 16 SDMA
    engines; dispatch is ~625ns each) and ordered by first compute use
    (W1 q/k cols + x cols 0-511 first).
  - attention pipeline: the scores matmul for step n+1 is emitted before
    the PV matmuls of step n so the Tile scheduler (priority = emission
    order) keeps the exp stream fed.
  - the last processed head is an even head (its softmax normalize needs
    no partition-shift DMA) and runs as two 512-wide i-passes so the
    final out-projection overlaps its exp stream; the very last projection
    blocks bridge the final normalize to keep PE warm; PSUM evacuations in
    the tail alternate DVE/ACT.
  - layouts as in the earlier fp32r version: x transposed on host, q/k
    produced transposed (qkT[f,s]), k zero-padded to K=128, v natural with
    a ones column per head so the PV matmul also yields softmax denominators.
  - bf16 for x, W1, q, k, v, exp(probs) and the y partials (halves DMA and
    SBUF; errors average out through the softmax weighted sum); fp32r for
    outT/W_proj so the final projection stays accurate. Measured output
    rel-inf error ~7e-3 vs f64 (gate 2e-2).
  - exp [128,1024] tiles straight out of PSUM, 1/sqrt(hd) scale fused, no
    max-subtraction (scores bounded for N(0,1)-scale inputs).
"""

import numpy as np

N_CORES = 8
B, S, D = 2, 2048, 1024
H, HD = 16, 64
HPC = 4          # heads per core
F_QK = 512      # q+k features per core (4 heads x 64 x 2)
F_V = 256       # v features per core
FT = 768        # total qkv features per core
SC = 512        # seq chunk (matmul N)
NSC = S // SC   # 4
NJ = S // 128   # 16 j-blocks
NDC = D // 128  # 8 contraction chunks

_CACHE = {}


def _build(repeat=1):
    import contextlib
    import concourse.bass as bass  # noqa: F401
    import concourse.mybir as mybir
    import concourse.tile as tile
    from concourse import bacc

    F32, F32R = mybir.dt.float32, mybir.dt.float32r
    BF16 = mybir.dt.bfloat16

    nc = bacc.Bacc("TRN2", target_bir_lowering=False, num_devices=N_CORES)
    xT = nc.declare_dram_parameter("xT", [D, S], BF16, isOutput=False)
    W1 = nc.declare_dram_parameter("W1", [D, FT], BF16, isOutput=False)
    b1 = nc.declare_dram_parameter("b1", [FT, 1], F32, isOutput=False)
    Wp = nc.declare_dram_parameter("Wp", [HPC * HD, D], F32R, isOutput=False)
    y = nc.declare_dram_parameter("y", [S, D], BF16, isOutput=True)

    with tile.TileContext(nc) as tc:
        with (
            tc.tile_pool(name="weights", bufs=1) as wpool,
            tc.tile_pool(name="persist", bufs=1) as persist,
            tc.tile_pool(name="xin", bufs=1) as xpool,
            tc.tile_pool(name="etile", bufs=6) as epool,
            tc.tile_pool(name="yout", bufs=3) as ypool,
            tc.tile_pool(name="small", bufs=3) as spool,
            tc.tile_pool(name="psS", bufs=2, space="PSUM") as psS,
            tc.tile_pool(name="psO", bufs=4, space="PSUM") as psO,
        ):
            # ---- inputs, ordered by when compute first needs them; chunks
            # of 4 dc-rows go out as single multi-dim-AP DMAs (each
            # InstDMACopy is split across all 16 SDMA engines, and dispatch
            # is ~625ns per instruction, so few + big wins).
            def rows_dma(dst, dst_w, src, r0, nr, src_c0, ncol, dst_c0=None):
                """dst[:, j*dst_w+dst_c0 :+ncol] <-
                src[(r0+j)*128:(r0+j+1)*128, src_c0:src_c0+ncol] per j"""
                if dst_c0 is None:
                    dst_c0 = src_c0
                s = src[r0 * 128:r0 * 128 + 1, 0:1]
                width = src.shape[-1]
                in_ap = bass.AP(tensor=s.tensor, offset=s.offset + src_c0,
                                ap=[[width, 128], [128 * width, nr], [1, ncol]])
                pp = dst.ap[0][0]
                out_ap = bass.AP(tensor=dst.tensor, offset=dst.offset + dst_c0,
                                 ap=[[pp, 128], [dst_w, nr], [1, ncol]])
                nc.sync.dma_start(out=out_ap, in_=in_ap)

            w1b = [wpool.tile([128, 4 * FT], BF16, tag=f"w1b_{g}",
                              name=f"w1b_{g}") for g in range(2)]
            w1t = [w1b[dc // 4][:, (dc % 4) * FT:(dc % 4 + 1) * FT]
                   for dc in range(NDC)]
            xb = [[xpool.tile([128, 4096], BF16, tag=f"xb_{pair}_{g}",
                              name=f"xb_{pair}_{g}") for g in range(2)]
                  for pair in range(2)]
            xts2 = [[xb[pair][dc // 4][:, (dc % 4) * 1024:(dc % 4 + 1) * 1024]
                     for dc in range(NDC)] for pair in range(2)]
            # need-order: W1 q+k cols / x cols 0-511 (h0 i 0-511 + k,v sc0),
            # then W1 v cols, then x cols 512-1023, then x cols 1024-2047.
            rows_dma(w1b[0], FT, W1, 0, 4, 0, 384)
            rows_dma(xb[0][0], 1024, xT, 0, 4, 0, 512)
            rows_dma(w1b[1], FT, W1, 4, 4, 0, 384)
            rows_dma(xb[0][1], 1024, xT, 4, 4, 0, 512)
            rows_dma(w1b[0], FT, W1, 0, 4, 384, 384)
            rows_dma(w1b[1], FT, W1, 4, 4, 384, 384)
            rows_dma(xb[0][0], 1024, xT, 0, 4, 512, 512)
            rows_dma(xb[0][1], 1024, xT, 4, 4, 512, 512)
            b6 = wpool.tile([128, 6], F32, tag="b6", name="b6")
            b1s = b1[0:128, 0:1]
            b6_ap = bass.AP(tensor=b1s.tensor, offset=b1s.offset,
                            ap=[[1, 128], [128, 6]])
            nc.sync.dma_start(out=b6, in_=b6_ap)
            bv = wpool.tile([128, F_V], F32, tag="bv", name="bv")
            bvsrc = b1[F_QK:FT, 0:1]
            bv_ap = bass.AP(tensor=bvsrc.tensor, offset=bvsrc.offset,
                            ap=[[0, 128], [1, F_V]])
            nc.sync.dma_start(out=bv, in_=bv_ap)
            ones = wpool.tile([128, 1], F32, tag="ones", name="ones")
            nc.vector.memset(ones, 1.0)
            rows_dma(xb[1][0], 1024, xT, 0, 4, 1024, 1024, dst_c0=0)
            rows_dma(xb[1][1], 1024, xT, 4, 4, 1024, 1024, dst_c0=0)
            wpt = []
            for p in range(2):
                t = wpool.tile([128, D], F32R, tag=f"wp_{p}", name=f"wp_{p}")
                nc.sync.dma_start(out=t, in_=Wp[p * 128:(p + 1) * 128, :])
                wpt.append(t)

            if repeat > 1:
                ET = mybir.EngineType
                loop_cm = tc.For_i(0, repeat, 1,
                                   hint_engines=(ET.PE, ET.DVE, ET.Activation,
                                                 ET.Pool, ET.SP))
            else:
                loop_cm = contextlib.nullcontext()
            with loop_cm:
                _emit_body(nc, tc, mybir, locals())
    nc.compile()
    return nc


def _emit_body(nc, tc, mybir, env):
    F32, F32R = mybir.dt.float32, mybir.dt.float32r
    BF16 = mybir.dt.bfloat16
    AF = mybir.ActivationFunctionType
    w1t, wpt, b6, bv, ones = (env[k] for k in ("w1t", "wpt", "b6", "bv", "ones"))
    xts2, y = env["xts2"], env["y"]
    wpool, persist, epool, ypool, spool = (
        env[k] for k in ("wpool", "persist", "epool", "ypool", "spool"))
    psS, psO = env["psS"], env["psO"]

    def xts(sc, dc):
        return xts2[sc // 2][dc][:, (sc % 2) * SC:(sc % 2 + 1) * SC]

    # persistent activation tiles. k tiles are NOT zero-padded: the scores
    # matmuls contract over K=64 row-tiles (head-even at partitions 0-63,
    # head-odd at 64-127), and the two heads of a pair run *concurrently*
    # in different PE row-groups.
    qk = [persist.tile([128, S], BF16, tag=f"qk_{p}", name=f"qk_{p}")
          for p in range(2)]
    kk = [persist.tile([128, S], BF16, tag=f"kk_{h}", name=f"kk_{h}")
          for h in range(HPC)]
    v4 = [persist.tile([128, HPC * (HD + 1)], BF16, tag=f"v4_{jc}",
                       name=f"v4_{jc}") for jc in range(NJ)]
    outT = [persist.tile([128, S], F32R, tag=f"outT_{p}", name=f"outT_{p}")
            for p in range(2)]

    def emit_qk_block(sc, fb):
        pq = psO.tile([128, SC], F32, tag="po", name="pq")
        for dc in range(NDC):
            nc.tensor.matmul(pq, w1t[dc][:, fb * 128:(fb + 1) * 128],
                             xts(sc, dc), start=(dc == 0), stop=(dc == NDC - 1))
        ssl1 = slice(sc * SC, (sc + 1) * SC)
        if fb < 2:
            nc.vector.tensor_scalar_add(qk[fb][:, ssl1], pq, b6[:, fb:fb + 1])
        else:
            ke, ko = kk[2 * (fb - 2)], kk[2 * (fb - 2) + 1]
            nc.vector.tensor_scalar_add(ke[0:64, ssl1], pq[0:64, :],
                                        b6[0:64, fb:fb + 1])
            nc.vector.tensor_scalar_add(ko[64:128, ssl1], pq[64:128, :],
                                        b6[64:128, fb:fb + 1])

    def emit_v_block(sc, sb):
        jc = sc * 4 + sb
        pv = psO.tile([128, F_V], F32, tag="po", name="pv")
        for dc in range(NDC):
            nc.tensor.matmul(pv, xts(sc, dc)[:, sb * 128:(sb + 1) * 128],
                             w1t[dc][:, F_QK:FT],
                             start=(dc == 0), stop=(dc == NDC - 1))
        for h in range(HPC):
            nc.vector.tensor_add(v4[jc][:, h * (HD + 1):h * (HD + 1) + HD],
                                 pv[:, h * HD:(h + 1) * HD],
                                 bv[:, h * HD:(h + 1) * HD])
            nc.vector.tensor_copy(
                v4[jc][:, h * (HD + 1) + HD:(h + 1) * (HD + 1)], ones)

    # STAGE 1, fully dense (back-to-back matmuls keep the PE clock-gate at
    # full rate): k + v for all chunks, then all q.
    for sc in range(NSC):
        emit_qk_block(sc, 2)
        emit_qk_block(sc, 3)
        for sb in range(4):
            emit_v_block(sc, sb)
    for fb in range(2):
        for sc in range(NSC):
            emit_qk_block(sc, fb)

    # ---- out-projection (dense blocks; merged 1024-wide y DMA per
    # s-block; PSUM evacuations alternate ACT/DVE — ACT is idle in the
    # projection phases) ----
    def emit_proj(sblk):
        ssl = slice(sblk * 128, (sblk + 1) * 128)
        ysb = ypool.tile([128, 1024], BF16, tag="ysb", name="ysb")
        for oc in range(2):
            osl = slice(oc * SC, (oc + 1) * SC)
            py = psO.tile([128, SC], F32, tag="po", name="py")
            nc.tensor.matmul(py, outT[0][:, ssl], wpt[0][:, osl],
                             start=True, stop=False)
            nc.tensor.matmul(py, outT[1][:, ssl], wpt[1][:, osl],
                             start=False, stop=True)
            if oc == 0:
                nc.scalar.copy(ysb[:, osl], py)
            else:
                nc.vector.tensor_copy(ysb[:, osl], py)
        nc.sync.dma_start(out=y[ssl, :], in_=ysb)

    # ---- attention: per (pair, i-chunk): both heads' scores run as
    # concurrent K=64 row-tiled matmuls; exp per head; PV per head (ones
    # column yields softmax denominators). ----
    def make_ss_pair(p, i0, jc):
        ssA = psS.tile([128, 1024], F32, tag="ss", name="ssA")
        ssB = psS.tile([128, 1024], F32, tag="ss", name="ssB")
        jsl = slice(jc * 128, (jc + 1) * 128)
        for half in range(2):
            cs = slice(i0 + half * SC, i0 + (half + 1) * SC)
            hs = slice(half * SC, (half + 1) * SC)
            nc.tensor.matmul(ssA[:, hs], kk[2 * p][0:64, jsl],
                             qk[p][0:64, cs], start=True, stop=True)
            nc.tensor.matmul(ssB[:, hs], kk[2 * p + 1][64:128, jsl],
                             qk[p][64:128, cs], start=True, stop=True)
        return ssA, ssB

    def normalize(h, i0, po):
        p = h // 2
        for half in range(2):
            isl = slice(i0 + half * SC, i0 + (half + 1) * SC)
            posb = spool.tile([HD + 1, SC], F32, tag="posb", name="posb")
            nc.vector.tensor_copy(posb, po[half])
            recip = spool.tile([1, SC], F32, tag="recip", name="recip")
            nc.vector.reciprocal(recip, posb[HD:HD + 1, :])
            rb = spool.tile([HD, SC], F32, tag="rb", name="rb")
            nc.gpsimd.partition_broadcast(rb, recip)
            if h % 2 == 0:
                nc.vector.tensor_mul(outT[p][0:HD, isl], posb[0:HD, :], rb)
            else:
                tmp = spool.tile([HD, SC], F32R, tag="tmp64", name="tmp64")
                nc.vector.tensor_mul(tmp, posb[0:HD, :], rb)
                nc.sync.dma_start(out=outT[p][HD:128, isl], in_=tmp)

    pairs = [(0, 0), (1, 0), (0, 1024), (1, 1024)]
    sspair = make_ss_pair(0, 0, 0)
    for idx, (p, i0) in enumerate(pairs):
        hA, hB = 2 * p, 2 * p + 1
        vslA = slice(hA * (HD + 1), (hA + 1) * (HD + 1))
        vslB = slice(hB * (HD + 1), (hB + 1) * (HD + 1))
        poA = [psO.tile([HD + 1, SC], F32, tag="po", name=f"poA_{half}")
               for half in range(2)]
        poB = [psO.tile([HD + 1, SC], F32, tag="po", name=f"poB_{half}")
               for half in range(2)]
        for jc in range(NJ):
            ssA, ssB = sspair
            exA = epool.tile([128, 1024], BF16, tag="ex", name="exA")
            nc.scalar.activation(exA, ssA, AF.Exp, bias=0.0, scale=0.125)
            exB = epool.tile([128, 1024], BF16, tag="ex", name="exB")
            nc.scalar.activation(exB, ssB, AF.Exp, bias=0.0, scale=0.125)
            if jc + 1 < NJ:
                sspair = make_ss_pair(p, i0, jc + 1)
            elif idx + 1 < len(pairs):
                sspair = make_ss_pair(*pairs[idx + 1], 0)
            for half in range(2):
                hs = slice(half * SC, (half + 1) * SC)
                nc.tensor.matmul(poA[half], v4[jc][:, vslA], exA[:, hs],
                                 start=(jc == 0), stop=(jc == NJ - 1))
            for half in range(2):
                hs = slice(half * SC, (half + 1) * SC)
                nc.tensor.matmul(poB[half], v4[jc][:, vslB], exB[:, hs],
                                 start=(jc == 0), stop=(jc == NJ - 1))
        # odd head first: its partition-shift DMA overlaps the even norm
        normalize(hB, i0, poB)
        normalize(hA, i0, poA)
        if idx == 1:
            for sblk in range(8):
                emit_proj(sblk)
        elif idx == 3:
            for sblk in range(8, 16):
                emit_proj(sblk)


def _shards(x, W_qkv, b_qkv, W_proj):
    """Build per-core input maps."""
    import ml_dtypes
    bf16 = ml_dtypes.bfloat16
    xTb = [np.ascontiguousarray(x[b].T.astype(bf16)) for b in range(B)]
    in_maps = []
    for c in range(N_CORES):
        b, g = c // 4, c % 4
        cols = slice(g * HPC * HD, (g + 1) * HPC * HD)  # 256 cols within q/k/v
        W1 = np.concatenate([W_qkv[:, 0 * D:1 * D][:, cols],
                             W_qkv[:, 1 * D:2 * D][:, cols],
                             W_qkv[:, 2 * D:3 * D][:, cols]], axis=1)
        b1 = np.concatenate([b_qkv[0 * D:1 * D][cols],
                             b_qkv[1 * D:2 * D][cols],
                             b_qkv[2 * D:3 * D][cols]]).reshape(FT, 1)
        Wp = W_proj[g * HPC * HD:(g + 1) * HPC * HD, :]
        in_maps.append({
            "xT": xTb[b],
            "W1": np.ascontiguousarray(W1.astype(bf16)),
            "b1": np.ascontiguousarray(b1, dtype=np.float32),
            "Wp": np.ascontiguousarray(Wp, dtype=np.float32),
        })
    return in_maps


def kernel(x, W_qkv, b_qkv, W_proj, b_proj):
    from concourse.bass_utils import run_bass_kernel_spmd

    x = np.asarray(x, dtype=np.float32)
    W_qkv = np.asarray(W_qkv, dtype=np.float32)
    b_qkv = np.asarray(b_qkv, dtype=np.float32)
    W_proj = np.asarray(W_proj, dtype=np.float32)
    b_proj = np.asarray(b_proj, dtype=np.float32)

    if "nc" not in _CACHE:
        _CACHE["nc"] = _build()
    nc = _CACHE["nc"]

    in_maps = _shards(x, W_qkv, b_qkv, W_proj)
    res = run_bass_kernel_spmd(nc, in_maps, list(range(N_CORES)), trace=False)

    out = np.empty((B, S, D), dtype=np.float32)
    for b in range(B):
        acc = res.results[4 * b]["y"].astype(np.float32)
        for g in range(1, 4):
            acc = acc + res.results[4 * b + g]["y"].astype(np.float32)
        out[b] = acc + b_proj[None, :]
    return out


if __name__ == "__main__":
    rng = np.random.default_rng(0)
    scale = 1.0 / np.sqrt(D)
    inputs = {
        "x": rng.standard_normal((B, S, D), dtype=np.float32),
        "W_qkv": (rng.standard_normal((D, 3 * D)).astype(np.float32) * scale),
        "b_qkv": np.zeros(3 * D, np.float32),
        "W_proj": (rng.standard_normal((D, D)).astype(np.float32) * scale),
        "b_proj": np.zeros(D, np.float32),
    }
    out = kernel(**inputs)
    print("out", out.shape, out.dtype, np.abs(out).max())


# revision 33
# speedup vs baseline: 1.1655x; 1.1655x over previous
"""Multi-head self-attention on 8 TRN2 NeuronCores.

Problem: x[2,2048,1024] -> qkv proj -> 16-head attention -> out proj.
Sharding: core c handles batch b=c//4 and head group g=c%4 (4 heads each).
Each core computes a partial output y_c[2048,1024] = attn_out_heads(g) @ W_proj[rows g];
host sums the 4 partials per batch and adds b_proj.

Design notes (from HW measurement, ~224 us/core vs 253 us fp32r baseline):
  - stage 1 (qkv projection) runs DENSE, back-to-back: spreading these
    matmuls thinly into the attention stream measured 1.5-2x slower on HW
    (PE HAM clock-gate drops to 1.2 GHz in sparse phases + cross-engine
    semaphore chains); only the q blocks for i 1024-2047 and the
    out-projection drip into the attention stream, as whole dense blocks.
  - input DMAs are few + big (one InstDMACopy splits over all 16 SDMA
    engines; dispatch is ~625ns each) and ordered by first compute use
    (W1 q/k cols + x cols 0-511 first).
  - attention pipeline: the scores matmul for step n+1 is emitted before
    the PV matmuls of step n so the Tile scheduler (priority = emission
    order) keeps the exp stream fed.
  - the last processed head is an even head (its softmax normalize needs
    no partition-shift DMA) and runs as two 512-wide i-passes so the
    final out-projection overlaps its exp stream; the very last projection
    blocks bridge the final normalize to keep PE warm; PSUM evacuations in
    the tail alternate DVE/ACT.
  - layouts as in the earlier fp32r version: x transposed on host, q/k
    produced transposed (qkT[f,s]), k zero-padded to K=128, v natural with
    a ones column per head so the PV matmul also yields softmax denominators.
  - bf16 for x, W1, q, k, v, exp(probs) and the y partials (halves DMA and
    SBUF; errors average out through the softmax weighted sum); fp32r for
    outT/W_proj so the final projection stays accurate. Measured output
    rel-inf error ~7e-3 vs f64 (gate 2e-2).
  - exp [128,1024] tiles straight out of PSUM, 1/sqrt(hd) scale fused, no
    max-subtraction (scores bounded for N(0,1)-scale inputs).
"""

import numpy as np

N_CORES = 8
B, S, D = 2, 2048, 1024
H, HD = 16, 64
HPC = 4          # heads per core
F_QK = 512      # q+k features per core (4 heads x 64 x 2)
F_V = 256       # v features per core
FT = 768        # total qkv features per core
SC = 512        # seq chunk (matmul N)
NSC = S // SC   # 4
NJ = S // 128   # 16 j-blocks
NDC = D // 128  # 8 contraction chunks

_CACHE = {}


def _build(repeat=1):
    import contextlib
    import concourse.bass as bass  # noqa: F401
    import concourse.mybir as mybir
    import concourse.tile as tile
    from concourse import bacc

    F32, F32R = mybir.dt.float32, mybir.dt.float32r
    BF16 = mybir.dt.bfloat16

    nc = bacc.Bacc("TRN2", target_bir_lowering=False, num_devices=N_CORES)
    xT = nc.declare_dram_parameter("xT", [D, S], BF16, isOutput=False)
    W1 = nc.declare_dram_parameter("W1", [D, FT], BF16, isOutput=False)
    b1 = nc.declare_dram_parameter("b1", [FT, 1], F32, isOutput=False)
    Wp = nc.declare_dram_parameter("Wp", [HPC * HD, D], F32R, isOutput=False)
    y = nc.declare_dram_parameter("y", [S, D], BF16, isOutput=True)

    with tile.TileContext(nc) as tc:
        with (
            tc.tile_pool(name="weights", bufs=1) as wpool,
            tc.tile_pool(name="persist", bufs=1) as persist,
            tc.tile_pool(name="xin", bufs=1) as xpool,
            tc.tile_pool(name="etile", bufs=6) as epool,
            tc.tile_pool(name="yout", bufs=3) as ypool,
            tc.tile_pool(name="small", bufs=3) as spool,
            tc.tile_pool(name="psA", bufs=2, space="PSUM") as psA,
            tc.tile_pool(name="psS", bufs=2, space="PSUM") as psS,
            tc.tile_pool(name="psO", bufs=2, space="PSUM") as psO,
        ):
            # ---- inputs, ordered by when compute first needs them; chunks
            # of 4 dc-rows go out as single multi-dim-AP DMAs (each
            # InstDMACopy is split across all 16 SDMA engines, and dispatch
            # is ~625ns per instruction, so few + big wins).
            def rows_dma(dst, dst_w, src, r0, nr, src_c0, ncol, dst_c0=None):
                """dst[:, j*dst_w+dst_c0 :+ncol] <-
                src[(r0+j)*128:(r0+j+1)*128, src_c0:src_c0+ncol] per j"""
                if dst_c0 is None:
                    dst_c0 = src_c0
                s = src[r0 * 128:r0 * 128 + 1, 0:1]
                width = src.shape[-1]
                in_ap = bass.AP(tensor=s.tensor, offset=s.offset + src_c0,
                                ap=[[width, 128], [128 * width, nr], [1, ncol]])
                pp = dst.ap[0][0]
                out_ap = bass.AP(tensor=dst.tensor, offset=dst.offset + dst_c0,
                                 ap=[[pp, 128], [dst_w, nr], [1, ncol]])
                nc.sync.dma_start(out=out_ap, in_=in_ap)

            w1b = [wpool.tile([128, 4 * FT], BF16, tag=f"w1b_{g}",
                              name=f"w1b_{g}") for g in range(2)]
            w1t = [w1b[dc // 4][:, (dc % 4) * FT:(dc % 4 + 1) * FT]
                   for dc in range(NDC)]
            xb = [[xpool.tile([128, 4096], BF16, tag=f"xb_{pair}_{g}",
                              name=f"xb_{pair}_{g}") for g in range(2)]
                  for pair in range(2)]
            xts2 = [[xb[pair][dc // 4][:, (dc % 4) * 1024:(dc % 4 + 1) * 1024]
                     for dc in range(NDC)] for pair in range(2)]
            # need-order: W1 q+k cols / x cols 0-511 (h0 i 0-511 + k,v sc0),
            # then W1 v cols, then x cols 512-1023, then x cols 1024-2047.
            rows_dma(w1b[0], FT, W1, 0, 4, 0, 384)
            rows_dma(xb[0][0], 1024, xT, 0, 4, 0, 512)
            rows_dma(w1b[1], FT, W1, 4, 4, 0, 384)
            rows_dma(xb[0][1], 1024, xT, 4, 4, 0, 512)
            rows_dma(w1b[0], FT, W1, 0, 4, 384, 384)
            rows_dma(w1b[1], FT, W1, 4, 4, 384, 384)
            rows_dma(xb[0][0], 1024, xT, 0, 4, 512, 512)
            rows_dma(xb[0][1], 1024, xT, 4, 4, 512, 512)
            b6 = wpool.tile([128, 6], F32, tag="b6", name="b6")
            b1s = b1[0:128, 0:1]
            b6_ap = bass.AP(tensor=b1s.tensor, offset=b1s.offset,
                            ap=[[1, 128], [128, 6]])
            nc.sync.dma_start(out=b6, in_=b6_ap)
            bv = wpool.tile([128, F_V], F32, tag="bv", name="bv")
            bvsrc = b1[F_QK:FT, 0:1]
            bv_ap = bass.AP(tensor=bvsrc.tensor, offset=bvsrc.offset,
                            ap=[[0, 128], [1, F_V]])
            nc.sync.dma_start(out=bv, in_=bv_ap)
            ones = wpool.tile([128, 1], F32, tag="ones", name="ones")
            nc.vector.memset(ones, 1.0)
            rows_dma(xb[1][0], 1024, xT, 0, 4, 1024, 1024, dst_c0=0)
            rows_dma(xb[1][1], 1024, xT, 4, 4, 1024, 1024, dst_c0=0)
            wpt = []
            for p in range(2):
                t = wpool.tile([128, D], F32R, tag=f"wp_{p}", name=f"wp_{p}")
                nc.sync.dma_start(out=t, in_=Wp[p * 128:(p + 1) * 128, :])
                wpt.append(t)

            if repeat > 1:
                ET = mybir.EngineType
                loop_cm = tc.For_i(0, repeat, 1,
                                   hint_engines=(ET.PE, ET.DVE, ET.Activation,
                                                 ET.Pool, ET.SP))
            else:
                loop_cm = contextlib.nullcontext()
            with loop_cm:
                _emit_body(nc, tc, mybir, locals())
    nc.compile()
    return nc


def _emit_body(nc, tc, mybir, env):
    from collections import deque

    F32, F32R = mybir.dt.float32, mybir.dt.float32r
    BF16 = mybir.dt.bfloat16
    AF = mybir.ActivationFunctionType
    w1t, wpt, b6, bv, ones = (env[k] for k in ("w1t", "wpt", "b6", "bv", "ones"))
    xts2, y = env["xts2"], env["y"]
    wpool, persist, epool, ypool, spool = (
        env[k] for k in ("wpool", "persist", "epool", "ypool", "spool"))
    psA, psS, psO = env["psA"], env["psS"], env["psO"]

    def xts(sc, dc):
        return xts2[sc // 2][dc][:, (sc % 2) * SC:(sc % 2 + 1) * SC]

    # persistent activation tiles
    qk = [persist.tile([128, S], BF16, tag=f"qk_{p}", name=f"qk_{p}")
          for p in range(2)]
    kpad = [persist.tile([128, S], BF16, tag=f"kpad_{h}", name=f"kpad_{h}")
            for h in range(HPC)]
    for h in range(HPC):
        zr = slice(64, 128) if h % 2 == 0 else slice(0, 64)
        nc.vector.memset(kpad[h].bitcast(F32)[zr, :], 0.0)
    v4 = [persist.tile([128, HPC * (HD + 1)], BF16, tag=f"v4_{jc}",
                       name=f"v4_{jc}") for jc in range(NJ)]
    outT = [persist.tile([128, S], F32R, tag=f"outT_{p}", name=f"outT_{p}")
            for p in range(2)]

    # ---- stage-1 emitters; drip-queued blocks are split into two 4-dc
    # halves so a single drained step stays ~2048 PE cycles ----
    def emit_q_half(sc, fb, half, state):
        if half == 0:
            state["pq"] = psA.tile([128, SC], F32, tag="mm", name="pq")
        pq = state["pq"]
        for dc in range(4 * half, 4 * half + 4):
            nc.tensor.matmul(pq, w1t[dc][:, fb * 128:(fb + 1) * 128],
                             xts(sc, dc), start=(dc == 0), stop=(dc == NDC - 1))
        if half == 1:
            ssl1 = slice(sc * SC, (sc + 1) * SC)
            if fb < 2:
                nc.vector.tensor_scalar_add(qk[fb][:, ssl1], pq,
                                            b6[:, fb:fb + 1])
            else:
                ke, ko = kpad[2 * (fb - 2)], kpad[2 * (fb - 2) + 1]
                nc.vector.tensor_scalar_add(ke[0:64, ssl1], pq[0:64, :],
                                            b6[0:64, fb:fb + 1])
                nc.vector.tensor_scalar_add(ko[64:128, ssl1], pq[64:128, :],
                                            b6[64:128, fb:fb + 1])

    def emit_qk_block(sc, fb):
        state = {}
        emit_q_half(sc, fb, 0, state)
        emit_q_half(sc, fb, 1, state)

    def emit_v_block(sc, sb):
        jc = sc * 4 + sb
        pv = psA.tile([128, F_V], F32, tag="mm", name="pv")
        for dc in range(NDC):
            nc.tensor.matmul(pv, xts(sc, dc)[:, sb * 128:(sb + 1) * 128],
                             w1t[dc][:, F_QK:FT],
                             start=(dc == 0), stop=(dc == NDC - 1))
        for h in range(HPC):
            nc.vector.tensor_add(v4[jc][:, h * (HD + 1):h * (HD + 1) + HD],
                                 pv[:, h * HD:(h + 1) * HD],
                                 bv[:, h * HD:(h + 1) * HD])
            nc.vector.tensor_copy(
                v4[jc][:, h * (HD + 1) + HD:(h + 1) * (HD + 1)], ones)

    # ---- out-projection (merged 1024-wide y DMA per s-block) ----
    def make_proj_steps(sblk, tail=False):
        ssl = slice(sblk * 128, (sblk + 1) * 128)
        state = {}

        def step(oc, use_act):
            if oc == 0:
                state["ysb"] = ypool.tile([128, 1024], BF16, tag="ysb",
                                          name="ysb")
            osl = slice(oc * SC, (oc + 1) * SC)
            py = psA.tile([128, SC], F32, tag="mm", name="py")
            nc.tensor.matmul(py, outT[0][:, ssl], wpt[0][:, osl],
                             start=True, stop=False)
            nc.tensor.matmul(py, outT[1][:, ssl], wpt[1][:, osl],
                             start=False, stop=True)
            if use_act:
                nc.scalar.copy(state["ysb"][:, osl], py)
            else:
                nc.vector.tensor_copy(state["ysb"][:, osl], py)
            if oc == 1:
                nc.sync.dma_start(out=y[ssl, :], in_=state["ysb"])
        # tail=True: ACT is idle after the last exp — split the PSUM
        # evacuations between DVE and ACT so neither serializes the tail
        return [lambda: step(0, tail), lambda: step(1, False)]

    # ---- drip queue ----
    workq = deque()

    def drain(n=1):
        for _ in range(n):
            if workq:
                workq.popleft()()

    def q_steps(sc, fb):
        # one dense 8-matmul block per step: interleaving finer-grained
        # work into the attention stream measures slower on HW
        return [lambda: emit_qk_block(sc, fb)]

    # STAGE 1, dense: back-to-back matmuls keep the PE HAM clock-gate at
    # full rate (spreading this work thinly into the attention stream
    # measured ~1.5-2x slower on HW). v + k for all chunks, then q for
    # i 0-1023; q for i 1024-2047 drips during attention.
    for sc in range(NSC):
        emit_qk_block(sc, 2)
        emit_qk_block(sc, 3)
        for sb in range(4):
            emit_v_block(sc, sb)
    emit_qk_block(0, 0)
    emit_qk_block(1, 0)
    emit_qk_block(0, 1)
    emit_qk_block(1, 1)

    workq.extend(q_steps(2, 0))                      # q pair0 i 1024-2047
    workq.extend(q_steps(3, 0))
    workq.extend(q_steps(2, 1))                      # q pair1 i 1024-2047
    workq.extend(q_steps(3, 1))

    # ---- attention stream ----
    # entries: (h-index, i0, width). The last processed head of the last
    # chunk is an even head (h=2: its normalize needs no partition-shift
    # DMA) and runs as two 512-wide passes so the final out-projection
    # overlaps its exp stream.
    entries = [(h, 0, 1024) for h in range(HPC)]
    entries += [(0, 1024, 1024), (1, 1024, 1024), (3, 1024, 1024),
                (2, 1024, 512), (2, 1536, 512)]

    def make_ss(ent, jc):
        h, i0, width = ent
        ss = psS.tile([128, width], F32, tag="ss", name="ss")
        for c0 in range(0, width, SC):
            nc.tensor.matmul(ss[:, c0:c0 + SC],
                             kpad[h][:, jc * 128:(jc + 1) * 128],
                             qk[h // 2][:, i0 + c0:i0 + c0 + SC],
                             start=True, stop=True)
        return ss

    def normalize(h, i0, width, po):
        p = h // 2
        for c0 in range(0, width, SC):
            isl = slice(i0 + c0, i0 + c0 + SC)
            posb = spool.tile([HD + 1, SC], F32, tag="posb", name="posb")
            nc.vector.tensor_copy(posb, po[c0 // SC])
            recip = spool.tile([1, SC], F32, tag="recip", name="recip")
            nc.vector.reciprocal(recip, posb[HD:HD + 1, :])
            rb = spool.tile([HD, SC], F32, tag="rb", name="rb")
            nc.gpsimd.partition_broadcast(rb, recip)
            if h % 2 == 0:
                nc.vector.tensor_mul(outT[p][0:HD, isl], posb[0:HD, :], rb)
            else:
                tmp = spool.tile([HD, SC], F32R, tag="tmp64", name="tmp64")
                nc.vector.tensor_mul(tmp, posb[0:HD, :], rb)
                nc.sync.dma_start(out=outT[p][HD:128, isl], in_=tmp)

    sstile = make_ss(entries[0], 0)
    for idx, ent in enumerate(entries):
        h, i0, width = ent
        vsl = slice(h * (HD + 1), (h + 1) * (HD + 1))
        po = [psO.tile([HD + 1, SC], F32, tag="po", name=f"po_{c0}")
              for c0 in range(0, width, SC)]
        for jc in range(NJ):
            ex = epool.tile([128, width], BF16, tag="ex", name="ex")
            nc.scalar.activation(ex, sstile, AF.Exp, bias=0.0, scale=0.125)
            # last entry: hold the remaining projection steps back so they
            # bridge the final normalize (keeps PE busy through the tail);
            # last 2 jc of each entry: keep PE clear for the boundary scores
            if idx < len(entries) - 1 and jc % 4 == 0 and jc < NJ - 2:
                drain(1)
            if jc + 1 < NJ:
                sstile = make_ss(ent, jc + 1)
            elif idx + 1 < len(entries):
                sstile = make_ss(entries[idx + 1], 0)
            for c0 in range(0, width, SC):
                nc.tensor.matmul(po[c0 // SC], v4[jc][:, vsl],
                                 ex[:, c0:c0 + SC],
                                 start=(jc == 0), stop=(jc == NJ - 1))
        normalize(h, i0, width, po)
        drain(2)
        # queue the projection for s-rows whose outT columns just completed
        if idx == 3:
            for sblk in range(8):
                workq.extend(make_proj_steps(sblk))
        elif idx == 7:
            for sblk in range(8, 12):
                workq.extend(make_proj_steps(sblk, tail=True))
        elif idx == 8:
            for sblk in range(12, 16):
                workq.extend(make_proj_steps(sblk, tail=True))
    while workq:
        workq.popleft()()


def _shards(x, W_qkv, b_qkv, W_proj):
    """Build per-core input maps."""
    import ml_dtypes
    bf16 = ml_dtypes.bfloat16
    xTb = [np.ascontiguousarray(x[b].T.astype(bf16)) for b in range(B)]
    in_maps = []
    for c in range(N_CORES):
        b, g = c // 4, c % 4
        cols = slice(g * HPC * HD, (g + 1) * HPC * HD)  # 256 cols within q/k/v
        W1 = np.concatenate([W_qkv[:, 0 * D:1 * D][:, cols],
                             W_qkv[:, 1 * D:2 * D][:, cols],
                             W_qkv[:, 2 * D:3 * D][:, cols]], axis=1)
        b1 = np.concatenate([b_qkv[0 * D:1 * D][cols],
                             b_qkv[1 * D:2 * D][cols],
                             b_qkv[2 * D:3 * D][cols]]).reshape(FT, 1)
        Wp = W_proj[g * HPC * HD:(g + 1) * HPC * HD, :]
        in_maps.append({
            "xT": xTb[b],
            "W1": np.ascontiguousarray(W1.astype(bf16)),
            "b1": np.ascontiguousarray(b1, dtype=np.float32),
            "Wp": np.ascontiguousarray(Wp, dtype=np.float32),
        })
    return in_maps


def kernel(x, W_qkv, b_qkv, W_proj, b_proj):
    from concourse.bass_utils import run_bass_kernel_spmd

    x = np.asarray(x, dtype=np.float32)
    W_qkv = np.asarray(W_qkv, dtype=np.float32)
    b_qkv = np.asarray(b_qkv, dtype=np.float32)
    W_proj = np.asarray(W_proj, dtype=np.float32)
    b_proj = np.asarray(b_proj, dtype=np.float32)

    if "nc" not in _CACHE:
        _CACHE["nc"] = _build()
    nc = _CACHE["nc"]

    in_maps = _shards(x, W_qkv, b_qkv, W_proj)
    res = run_bass_kernel_spmd(nc, in_maps, list(range(N_CORES)), trace=False)

    out = np.empty((B, S, D), dtype=np.float32)
    for b in range(B):
        acc = res.results[4 * b]["y"].astype(np.float32)
        for g in range(1, 4):
            acc = acc + res.results[4 * b + g]["y"].astype(np.float32)
        out[b] = acc + b_proj[None, :]
    return out


if __name__ == "__main__":
    rng = np.random.default_rng(0)
    scale = 1.0 / np.sqrt(D)
    inputs = {
        "x": rng.standard_normal((B, S, D), dtype=np.float32),
        "W_qkv": (rng.standard_normal((D, 3 * D)).astype(np.float32) * scale),
        "b_qkv": np.zeros(3 * D, np.float32),
        "W_proj": (rng.standard_normal((D, D)).astype(np.float32) * scale),
        "b_proj": np.zeros(D, np.float32),
    }
    out = kernel(**inputs)
    print("out", out.shape, out.dtype, np.abs(out).max())


# revision 34
# speedup vs baseline: 1.3796x; 1.1837x over previous
"""Multi-head self-attention on 8 TRN2 NeuronCores.

Problem: x[2,2048,1024] -> qkv proj -> 16-head attention -> out proj.
Sharding: core c handles batch b=c//4 and head group g=c%4 (4 heads each).
Each core computes a partial output y_c[2048,1024] = attn_out_heads(g) @ W_proj[rows g];
host sums the 4 partials per batch and adds b_proj.

Design notes (from HW measurement, ~224 us/core vs 253 us fp32r baseline):
  - stage 1 (qkv projection) runs DENSE, back-to-back: spreading these
    matmuls thinly into the attention stream measured 1.5-2x slower on HW
    (PE HAM clock-gate drops to 1.2 GHz in sparse phases + cross-engine
    semaphore chains); only the q blocks for i 1024-2047 and the
    out-projection drip into the attention stream, as whole dense blocks.
  - input DMAs are few + big (one InstDMACopy splits over all 16 SDMA
    engines; dispatch is ~625ns each) and ordered by first compute use
    (W1 q/k cols + x cols 0-511 first).
  - attention pipeline: the scores matmul for step n+1 is emitted before
    the PV matmuls of step n so the Tile scheduler (priority = emission
    order) keeps the exp stream fed.
  - the last processed head is an even head (its softmax normalize needs
    no partition-shift DMA) and runs as two 512-wide i-passes so the
    final out-projection overlaps its exp stream; the very last projection
    blocks bridge the final normalize to keep PE warm; PSUM evacuations in
    the tail alternate DVE/ACT.
  - layouts as in the earlier fp32r version: x transposed on host, q/k
    produced transposed (qkT[f,s]), k zero-padded to K=128, v natural with
    a ones column per head so the PV matmul also yields softmax denominators.
  - bf16 for x, W1, q, k, v, exp(probs) and the y partials (halves DMA and
    SBUF; errors average out through the softmax weighted sum); fp32r for
    outT/W_proj so the final projection stays accurate. Measured output
    rel-inf error ~7e-3 vs f64 (gate 2e-2).
  - exp [128,1024] tiles straight out of PSUM, 1/sqrt(hd) scale fused, no
    max-subtraction (scores bounded for N(0,1)-scale inputs).
"""

import numpy as np

N_CORES = 8
B, S, D = 2, 2048, 1024
H, HD = 16, 64
HPC = 4          # heads per core
F_QK = 512      # q+k features per core (4 heads x 64 x 2)
F_V = 256       # v features per core
FT = 768        # total qkv features per core
SC = 512        # seq chunk (matmul N)
NSC = S // SC   # 4
NJ = S // 128   # 16 j-blocks
NDC = D // 128  # 8 contraction chunks

_CACHE = {}


def _build(repeat=1):
    import contextlib
    import concourse.bass as bass  # noqa: F401
    import concourse.mybir as mybir
    import concourse.tile as tile
    from concourse import bacc

    F32, F32R = mybir.dt.float32, mybir.dt.float32r
    BF16 = mybir.dt.bfloat16

    nc = bacc.Bacc("TRN2", target_bir_lowering=False, num_devices=N_CORES)
    xT = nc.declare_dram_parameter("xT", [D, S], BF16, isOutput=False)
    W1 = nc.declare_dram_parameter("W1", [D, FT], BF16, isOutput=False)
    b1 = nc.declare_dram_parameter("b1", [FT, 1], F32, isOutput=False)
    Wp = nc.declare_dram_parameter("Wp", [HPC * HD, D], F32R, isOutput=False)
    y = nc.declare_dram_parameter("y", [S, D], BF16, isOutput=True)

    with tile.TileContext(nc) as tc:
        with (
            tc.tile_pool(name="weights", bufs=1) as wpool,
            tc.tile_pool(name="persist", bufs=1) as persist,
            tc.tile_pool(name="xin", bufs=1) as xpool,
            tc.tile_pool(name="etile", bufs=6) as epool,
            tc.tile_pool(name="yout", bufs=3) as ypool,
            tc.tile_pool(name="small", bufs=3) as spool,
            tc.tile_pool(name="psA", bufs=2, space="PSUM") as psA,
            tc.tile_pool(name="psS", bufs=2, space="PSUM") as psS,
            tc.tile_pool(name="psO", bufs=2, space="PSUM") as psO,
        ):
            # ---- inputs, ordered by when compute first needs them; chunks
            # of 4 dc-rows go out as single multi-dim-AP DMAs (each
            # InstDMACopy is split across all 16 SDMA engines, and dispatch
            # is ~625ns per instruction, so few + big wins).
            def rows_dma(dst, dst_w, src, r0, nr, src_c0, ncol, dst_c0=None):
                """dst[:, j*dst_w+dst_c0 :+ncol] <-
                src[(r0+j)*128:(r0+j+1)*128, src_c0:src_c0+ncol] per j"""
                if dst_c0 is None:
                    dst_c0 = src_c0
                s = src[r0 * 128:r0 * 128 + 1, 0:1]
                width = src.shape[-1]
                in_ap = bass.AP(tensor=s.tensor, offset=s.offset + src_c0,
                                ap=[[width, 128], [128 * width, nr], [1, ncol]])
                pp = dst.ap[0][0]
                out_ap = bass.AP(tensor=dst.tensor, offset=dst.offset + dst_c0,
                                 ap=[[pp, 128], [dst_w, nr], [1, ncol]])
                nc.sync.dma_start(out=out_ap, in_=in_ap)

            w1b = [wpool.tile([128, 4 * FT], BF16, tag=f"w1b_{g}",
                              name=f"w1b_{g}") for g in range(2)]
            w1t = [w1b[dc // 4][:, (dc % 4) * FT:(dc % 4 + 1) * FT]
                   for dc in range(NDC)]
            xb = [[xpool.tile([128, 4096], BF16, tag=f"xb_{pair}_{g}",
                              name=f"xb_{pair}_{g}") for g in range(2)]
                  for pair in range(2)]
            xts2 = [[xb[pair][dc // 4][:, (dc % 4) * 1024:(dc % 4 + 1) * 1024]
                     for dc in range(NDC)] for pair in range(2)]
            # need-order: W1 q+k cols / x cols 0-511 (h0 i 0-511 + k,v sc0),
            # then W1 v cols, then x cols 512-1023, then x cols 1024-2047.
            rows_dma(w1b[0], FT, W1, 0, 4, 0, 384)
            rows_dma(xb[0][0], 1024, xT, 0, 4, 0, 512)
            rows_dma(w1b[1], FT, W1, 4, 4, 0, 384)
            rows_dma(xb[0][1], 1024, xT, 4, 4, 0, 512)
            rows_dma(w1b[0], FT, W1, 0, 4, 384, 384)
            rows_dma(w1b[1], FT, W1, 4, 4, 384, 384)
            rows_dma(xb[0][0], 1024, xT, 0, 4, 512, 512)
            rows_dma(xb[0][1], 1024, xT, 4, 4, 512, 512)
            b6 = wpool.tile([128, 6], F32, tag="b6", name="b6")
            b1s = b1[0:128, 0:1]
            b6_ap = bass.AP(tensor=b1s.tensor, offset=b1s.offset,
                            ap=[[1, 128], [128, 6]])
            nc.sync.dma_start(out=b6, in_=b6_ap)
            bv = wpool.tile([128, F_V], F32, tag="bv", name="bv")
            bvsrc = b1[F_QK:FT, 0:1]
            bv_ap = bass.AP(tensor=bvsrc.tensor, offset=bvsrc.offset,
                            ap=[[0, 128], [1, F_V]])
            nc.sync.dma_start(out=bv, in_=bv_ap)
            ones = wpool.tile([128, 1], F32, tag="ones", name="ones")
            nc.vector.memset(ones, 1.0)
            rows_dma(xb[1][0], 1024, xT, 0, 4, 1024, 1024, dst_c0=0)
            rows_dma(xb[1][1], 1024, xT, 4, 4, 1024, 1024, dst_c0=0)
            wpt = []
            for p in range(2):
                t = wpool.tile([128, D], F32R, tag=f"wp_{p}", name=f"wp_{p}")
                nc.sync.dma_start(out=t, in_=Wp[p * 128:(p + 1) * 128, :])
                wpt.append(t)

            if repeat > 1:
                ET = mybir.EngineType
                loop_cm = tc.For_i(0, repeat, 1,
                                   hint_engines=(ET.PE, ET.DVE, ET.Activation,
                                                 ET.Pool, ET.SP))
            else:
                loop_cm = contextlib.nullcontext()
            with loop_cm:
                _emit_body(nc, tc, mybir, locals())
    nc.compile()
    return nc


def _emit_body(nc, tc, mybir, env):
    from collections import deque

    F32, F32R = mybir.dt.float32, mybir.dt.float32r
    BF16 = mybir.dt.bfloat16
    AF = mybir.ActivationFunctionType
    w1t, wpt, b6, bv, ones = (env[k] for k in ("w1t", "wpt", "b6", "bv", "ones"))
    xts2, y = env["xts2"], env["y"]
    wpool, persist, epool, ypool, spool = (
        env[k] for k in ("wpool", "persist", "epool", "ypool", "spool"))
    psA, psS, psO = env["psA"], env["psS"], env["psO"]

    def xts(sc, dc):
        return xts2[sc // 2][dc][:, (sc % 2) * SC:(sc % 2 + 1) * SC]

    # persistent activation tiles
    qk = [persist.tile([128, S], BF16, tag=f"qk_{p}", name=f"qk_{p}")
          for p in range(2)]
    kpad = [persist.tile([128, S], BF16, tag=f"kpad_{h}", name=f"kpad_{h}")
            for h in range(HPC)]
    for h in range(HPC):
        zr = slice(64, 128) if h % 2 == 0 else slice(0, 64)
        nc.vector.memset(kpad[h].bitcast(F32)[zr, :], 0.0)
    v4 = [persist.tile([128, HPC * (HD + 1)], BF16, tag=f"v4_{jc}",
                       name=f"v4_{jc}") for jc in range(NJ)]
    outT = [persist.tile([128, S], F32R, tag=f"outT_{p}", name=f"outT_{p}")
            for p in range(2)]

    # ---- stage-1 emitters; drip-queued blocks are split into two 4-dc
    # halves so a single drained step stays ~2048 PE cycles ----
    def emit_q_half(sc, fb, half, state):
        if half == 0:
            state["pq"] = psA.tile([128, SC], F32, tag="mm", name="pq")
        pq = state["pq"]
        for dc in range(4 * half, 4 * half + 4):
            nc.tensor.matmul(pq, w1t[dc][:, fb * 128:(fb + 1) * 128],
                             xts(sc, dc), start=(dc == 0), stop=(dc == NDC - 1))
        if half == 1:
            ssl1 = slice(sc * SC, (sc + 1) * SC)
            if fb < 2:
                nc.vector.tensor_scalar_add(qk[fb][:, ssl1], pq,
                                            b6[:, fb:fb + 1])
            else:
                ke, ko = kpad[2 * (fb - 2)], kpad[2 * (fb - 2) + 1]
                nc.vector.tensor_scalar_add(ke[0:64, ssl1], pq[0:64, :],
                                            b6[0:64, fb:fb + 1])
                nc.vector.tensor_scalar_add(ko[64:128, ssl1], pq[64:128, :],
                                            b6[64:128, fb:fb + 1])

    def emit_qk_block(sc, fb):
        state = {}
        emit_q_half(sc, fb, 0, state)
        emit_q_half(sc, fb, 1, state)

    def emit_v_block(sc, sb):
        jc = sc * 4 + sb
        pv = psA.tile([128, F_V], F32, tag="mm", name="pv")
        for dc in range(NDC):
            nc.tensor.matmul(pv, xts(sc, dc)[:, sb * 128:(sb + 1) * 128],
                             w1t[dc][:, F_QK:FT],
                             start=(dc == 0), stop=(dc == NDC - 1))
        for h in range(HPC):
            nc.vector.tensor_add(v4[jc][:, h * (HD + 1):h * (HD + 1) + HD],
                                 pv[:, h * HD:(h + 1) * HD],
                                 bv[:, h * HD:(h + 1) * HD])
            nc.vector.tensor_copy(
                v4[jc][:, h * (HD + 1) + HD:(h + 1) * (HD + 1)], ones)

    # ---- out-projection (merged 1024-wide y DMA per s-block) ----
    def make_proj_steps(sblk, tail=False):
        ssl = slice(sblk * 128, (sblk + 1) * 128)
        state = {}

        def step(oc, use_act):
            if oc == 0:
                state["ysb"] = ypool.tile([128, 1024], BF16, tag="ysb",
                                          name="ysb")
            osl = slice(oc * SC, (oc + 1) * SC)
            py = psA.tile([128, SC], F32, tag="mm", name="py")
            nc.tensor.matmul(py, outT[0][:, ssl], wpt[0][:, osl],
                             start=True, stop=False)
            nc.tensor.matmul(py, outT[1][:, ssl], wpt[1][:, osl],
                             start=False, stop=True)
            if use_act:
                nc.scalar.copy(state["ysb"][:, osl], py)
            else:
                nc.vector.tensor_copy(state["ysb"][:, osl], py)
            if oc == 1:
                nc.sync.dma_start(out=y[ssl, :], in_=state["ysb"])
        # tail=True: ACT is idle after the last exp — split the PSUM
        # evacuations between DVE and ACT so neither serializes the tail
        return [lambda: step(0, tail), lambda: step(1, False)]

    # ---- drip queue ----
    workq = deque()

    def drain(n=1):
        for _ in range(n):
            if workq:
                workq.popleft()()

    def q_steps(sc, fb):
        # one dense 8-matmul block per step: interleaving finer-grained
        # work into the attention stream measures slower on HW
        return [lambda: emit_qk_block(sc, fb)]

    # STAGE 1, dense: back-to-back matmuls keep the PE HAM clock-gate at
    # full rate (spreading this work thinly into the attention stream
    # measured ~1.5-2x slower on HW). v + k for all chunks, then q for
    # i 0-1023; q for i 1024-2047 drips during attention.
    for sc in range(NSC):
        emit_qk_block(sc, 2)
        emit_qk_block(sc, 3)
        for sb in range(4):
            emit_v_block(sc, sb)
    emit_qk_block(0, 0)
    emit_qk_block(1, 0)
    emit_qk_block(0, 1)
    emit_qk_block(1, 1)

    workq.extend(q_steps(2, 0))                      # q pair0 i 1024-2047
    workq.extend(q_steps(3, 0))
    workq.extend(q_steps(2, 1))                      # q pair1 i 1024-2047
    workq.extend(q_steps(3, 1))

    # ---- attention stream ----
    # entries: (h-index, i0, width). The last processed head of the last
    # chunk is an even head (h=2: its normalize needs no partition-shift
    # DMA) and runs as two 512-wide passes so the final out-projection
    # overlaps its exp stream.
    entries = [(h, 0, 1024) for h in range(HPC)]
    entries += [(0, 1024, 1024), (1, 1024, 1024), (3, 1024, 1024),
                (2, 1024, 512), (2, 1536, 512)]

    def make_ss_half(ent, jc, c0):
        # 512-wide micro-steps with a 4-deep ss rotation (same 4 PSUM banks
        # as 2x1024): shorter stages double the pipeline depth and absorb
        # cross-engine semaphore latency between scores/exp/PV
        h, i0, width = ent
        ss = psS.tile([128, SC], F32, tag="ss", bufs=4, name="ss")
        nc.tensor.matmul(ss, kpad[h][:, jc * 128:(jc + 1) * 128],
                         qk[h // 2][:, i0 + c0:i0 + c0 + SC],
                         start=True, stop=True)
        return ss

    def normalize(h, i0, width, po):
        p = h // 2
        for c0 in range(0, width, SC):
            isl = slice(i0 + c0, i0 + c0 + SC)
            posb = spool.tile([HD + 1, SC], F32, tag="posb", name="posb")
            nc.vector.tensor_copy(posb, po[c0 // SC])
            recip = spool.tile([1, SC], F32, tag="recip", name="recip")
            nc.vector.reciprocal(recip, posb[HD:HD + 1, :])
            rb = spool.tile([HD, SC], F32, tag="rb", name="rb")
            nc.gpsimd.partition_broadcast(rb, recip)
            if h % 2 == 0:
                nc.vector.tensor_mul(outT[p][0:HD, isl], posb[0:HD, :], rb)
            else:
                tmp = spool.tile([HD, SC], F32R, tag="tmp64", name="tmp64")
                nc.vector.tensor_mul(tmp, posb[0:HD, :], rb)
                nc.sync.dma_start(out=outT[p][HD:128, isl], in_=tmp)

    msteps = []
    for idx, ent in enumerate(entries):
        for jc in range(NJ):
            for c0 in range(0, ent[2], SC):
                msteps.append((idx, jc, c0))

    po_by_idx = {}
    sstile = make_ss_half(entries[0], 0, 0)
    for mi, (idx, jc, c0) in enumerate(msteps):
        h, i0, width = entries[idx]
        if jc == 0 and c0 == 0:
            po_by_idx[idx] = [psO.tile([HD + 1, SC], F32, tag="po",
                                       name=f"po_{cc}")
                              for cc in range(0, width, SC)]
        ex = epool.tile([128, SC], BF16, tag="ex", bufs=8, name="ex")
        nc.scalar.activation(ex, sstile, AF.Exp, bias=0.0, scale=0.125)
        # last entry: hold the remaining projection steps back so they
        # bridge the final normalize (keeps PE busy through the tail)
        if idx < len(entries) - 1 and jc % 4 == 0 and c0 == 0 and jc < NJ - 2:
            drain(1)
        if mi + 1 < len(msteps):
            nidx, njc, nc0 = msteps[mi + 1]
            sstile = make_ss_half(entries[nidx], njc, nc0)
        nc.tensor.matmul(po_by_idx[idx][c0 // SC],
                         v4[jc][:, h * (HD + 1):(h + 1) * (HD + 1)], ex,
                         start=(jc == 0), stop=(jc == NJ - 1))
        if jc == NJ - 1 and c0 == width - SC:
            normalize(h, i0, width, po_by_idx.pop(idx))
            drain(2)
            # queue projection for s-rows whose outT columns just completed
            if idx == 3:
                for sblk in range(8):
                    workq.extend(make_proj_steps(sblk))
            elif idx == 7:
                for sblk in range(8, 12):
                    workq.extend(make_proj_steps(sblk, tail=True))
            elif idx == 8:
                for sblk in range(12, 16):
                    workq.extend(make_proj_steps(sblk, tail=True))
    while workq:
        workq.popleft()()


def _shards(x, W_qkv, b_qkv, W_proj):
    """Build per-core input maps."""
    import ml_dtypes
    bf16 = ml_dtypes.bfloat16
    xTb = [np.ascontiguousarray(x[b].T.astype(bf16)) for b in range(B)]
    in_maps = []
    for c in range(N_CORES):
        b, g = c // 4, c % 4
        cols = slice(g * HPC * HD, (g + 1) * HPC * HD)  # 256 cols within q/k/v
        W1 = np.concatenate([W_qkv[:, 0 * D:1 * D][:, cols],
                             W_qkv[:, 1 * D:2 * D][:, cols],
                             W_qkv[:, 2 * D:3 * D][:, cols]], axis=1)
        b1 = np.concatenate([b_qkv[0 * D:1 * D][cols],
                             b_qkv[1 * D:2 * D][cols],
                             b_qkv[2 * D:3 * D][cols]]).reshape(FT, 1)
        Wp = W_proj[g * HPC * HD:(g + 1) * HPC * HD, :]
        in_maps.append({
            "xT": xTb[b],
            "W1": np.ascontiguousarray(W1.astype(bf16)),
            "b1": np.ascontiguousarray(b1, dtype=np.float32),
            "Wp": np.ascontiguousarray(Wp, dtype=np.float32),
        })
    return in_maps


def kernel(x, W_qkv, b_qkv, W_proj, b_proj):
    from concourse.bass_utils import run_bass_kernel_spmd

    x = np.asarray(x, dtype=np.float32)
    W_qkv = np.asarray(W_qkv, dtype=np.float32)
    b_qkv = np.asarray(b_qkv, dtype=np.float32)
    W_proj = np.asarray(W_proj, dtype=np.float32)
    b_proj = np.asarray(b_proj, dtype=np.float32)

    if "nc" not in _CACHE:
        _CACHE["nc"] = _build()
    nc = _CACHE["nc"]

    in_maps = _shards(x, W_qkv, b_qkv, W_proj)
    res = run_bass_kernel_spmd(nc, in_maps, list(range(N_CORES)), trace=False)

    out = np.empty((B, S, D), dtype=np.float32)
    for b in range(B):
        acc = res.results[4 * b]["y"].astype(np.float32)
        for g in range(1, 4):
            acc = acc + res.results[4 * b + g]["y"].astype(np.float32)
        out[b] = acc + b_proj[None, :]
    return out


if __name__ == "__main__":
    rng = np.random.default_rng(0)
    scale = 1.0 / np.sqrt(D)
    inputs = {
        "x": rng.standard_normal((B, S, D), dtype=np.float32),
        "W_qkv": (rng.standard_normal((D, 3 * D)).astype(np.float32) * scale),
        "b_qkv": np.zeros(3 * D, np.float32),
        "W_proj": (rng.standard_normal((D, D)).astype(np.float32) * scale),
        "b_proj": np.zeros(D, np.float32),
    }
    out = kernel(**inputs)
    print("out", out.shape, out.dtype, np.abs(out).max())
